# revision 1
# baseline (speedup 1.0000x reference)
"""MLA-style attention kernel for 8 TRN2 NeuronCores.

Sharding: core c -> batch b = c//4, heads r*4..r*4+3 where r = c%4.
Each core computes its batch's latent projections (duplicated within the
4-core group), its 4 heads' attention, and a partial output projection.
Partial outputs (transposed, [C, T]) are summed per batch on the host.

All activations on-chip use a transposed [feature, T] layout so the whole
matmul chain needs no inter-layer transposes; x and the weights are
transposed once on-chip via the PE array.  Matmuls run as float32r
(4x fp32 rate).  RoPE halves are kept planar (re rows 0:32, im rows
32:64, same permutation for q and k) which leaves dot products invariant;
the cos/sin tables are stored duplicated on both partition halves so
every DVE operand pair shares a base partition.  Scores are computed
pre-transposed (S^T tiles [k, q]) so exp writes P^T directly and the PV
matmul needs no on-chip transposes.  Causal softmax skips upper-triangle
512-blocks; diagonal blocks get an additive -1e30 mask before exp.  Softmax denominators
are accumulated with a ones-column matmul on the transposed probability
tiles and applied via a rank-1 broadcast matmul + DVE multiply.
"""
import math
import numpy as np

import concourse.bass as bass
import concourse.bacc as bacc
import concourse.mybir as mybir
import concourse.tile as tile
from concourse.bass_utils import run_bass_kernel_spmd

F32 = mybir.dt.float32
F32R = mybir.dt.float32r
BF16 = mybir.dt.bfloat16
Exp = mybir.ActivationFunctionType.Exp
Copy = mybir.ActivationFunctionType.Copy

B, T, C = 2, 2048, 2048
H = 16
HS = 128
NL = 512
RHD = 64
HLOC = 4              # heads per core
P = 128
NNL = NL // P         # 4
TCH = 512
NCH = T // TCH        # 4 chunks of T
NCS = C // TCH        # 4 c-strips for the down projection
SCALE = 1.0 / math.sqrt(HS + RHD)
NEG = -1.0e30

_NC_CACHE = {}


def _r(ap):
    return ap.bitcast(F32R)


def _deint(ap2d):
    # [p, 2d] -> (evens [p, d], odds [p, d]) along the free dim
    rr = ap2d.rearrange("p (d two) -> p two d", two=2)
    return rr[:, 0, :], rr[:, 1, :]


def build():
    nc = bacc.Bacc("TRN2", target_bir_lowering=False, debug=False, num_devices=8)

    x_ext = nc.dram_tensor("x", [TCH, C], F32R, kind="ExternalInput")
    wdq_ext = nc.dram_tensor("wdq", [NL, C], F32R, kind="ExternalInput")
    wdkv_ext = nc.dram_tensor("wdkv", [NL, C], F32R, kind="ExternalInput")
    wkr_ext = nc.dram_tensor("wkr", [RHD, C], F32R, kind="ExternalInput")
    wuq_ext = nc.dram_tensor("wuq", [HLOC * HS, NL], F32R, kind="ExternalInput")
    wuk_ext = nc.dram_tensor("wuk", [HLOC * HS, NL], F32R, kind="ExternalInput")
    wuv_ext = nc.dram_tensor("wuv", [HLOC * HS, NL], F32R, kind="ExternalInput")
    wqr_ext = nc.dram_tensor("wqr", [HLOC * RHD, NL], F32R, kind="ExternalInput")
    wo_ext = nc.dram_tensor("wo", [C, HLOC * HS], F32R, kind="ExternalInput")
    cos_ext = nc.dram_tensor("cos", [T, RHD // 2], F32R, kind="ExternalInput")
    sin_ext = nc.dram_tensor("sin", [T, RHD // 2], F32R, kind="ExternalInput")
    out_ext = nc.dram_tensor("out", [C, T], F32, kind="ExternalOutput")

    ident_dram = nc.inline_tensor(np.eye(P, dtype=np.float32), name="identc")
    ones_dram = nc.inline_tensor(np.ones((P, P), dtype=np.float32), name="onesc")
    # transposed sliding causal mask for S^T tiles [k-sub, q-chunk]:
    # m[jj, u] = 0 if u >= 384 + jj else -1e30.  For k-subtile ks the
    # diagonal-block mask is m[:, 384-128*ks : 384-128*ks+512], which allows
    # q-col qq >= ks*128 + jj.
    m = np.full((P, 896), NEG, dtype=np.float32)
    for jj in range(P):
        m[jj, 384 + jj:] = 0.0
    masks_dram = nc.inline_tensor(m, name="maskc")

    ahT_dram = nc.dram_tensor("ahT", [HLOC, HS, T], BF16)
    agin_dram = nc.dram_tensor("agin", [NL + NL + RHD, TCH], BF16)
    agout_dram = nc.dram_tensor("agout", [4, NL + NL + RHD, TCH], BF16)
    woT_dram = nc.dram_tensor("woT", [HLOC, P, C], BF16)

    with tile.TileContext(nc) as tc:
        with (
            tc.tile_pool(name="pers", bufs=1) as pers,
            tc.tile_pool(name="pmm", bufs=4, space="PSUM") as pmm,
            tc.tile_pool(name="ptp", bufs=2, space="PSUM") as ptp,
            tc.tile_pool(name="pou", bufs=1, space="PSUM") as pou,
        ):
            ident = pers.tile([P, P], F32R, tag="ident", name="ident")
            nc.sync.dma_start(ident[:], ident_dram.ap().bitcast(F32R))
            onesb = pers.tile([P, P], F32R, tag="onesb", name="onesb")
            nc.sync.dma_start(onesb[:], ones_dram.ap().bitcast(F32R))
            maskbuf = pers.tile([P, 896], BF16, tag="maskbuf", name="maskbuf")
            nc.gpsimd.dma_start(out=maskbuf[:], in_=masks_dram.ap())

            cqT = [pers.tile([P, T], BF16, tag=f"cqT{i}", name=f"cqT{i}")
                   for i in range(NNL)]
            ckvT = [pers.tile([P, T], BF16, tag=f"ckvT{i}", name=f"ckvT{i}")
                    for i in range(NNL)]
            kr = pers.tile([RHD, T], F32R, tag="kr", name="kr")
            ca = pers.tile([RHD, T], BF16, tag="ca", name="ca")
            sa = pers.tile([RHD, T], BF16, tag="sa", name="sa")

            def transpose_into(dst_ap, src_ap, eng="dve"):
                """PE-transpose src [p, w<=128] -> psum [w, p] -> copy to dst."""
                tp = ptp.tile([P, P], src_ap.dtype, tag="tp", name="tp")
                kdim = src_ap.shape[0]
                nc.tensor.transpose(
                    tp[: src_ap.shape[1], :kdim], src_ap, ident[:kdim, :kdim]
                )
                cp = nc.scalar.copy if eng == "act" else nc.vector.tensor_copy
                cp(dst_ap, tp[: src_ap.shape[1], :kdim])

            def transpose_pair_into(dst_ap, srcA, srcB, eng="dve"):
                """Two PE transposes into one psum tile, one 256-wide copy."""
                tp2 = ptp.tile([P, 2 * P], srcA.dtype, tag="tp", name="tp")
                nc.tensor.transpose(tp2[:, 0:P], srcA, ident[:])
                nc.tensor.transpose(tp2[:, P:2 * P], srcB, ident[:])
                cp = nc.scalar.copy if eng == "act" else nc.vector.tensor_copy
                cp(dst_ap, tp2[:])

            def rope(dst, dst_sl, raw, tmp, sl):
                """dst[:, dst_sl] = rope(raw) with planar re/im halves.

                raw may be PSUM or SBUF; all operand pairs share a base
                partition (tables are duplicated on both halves).
                """
                nc.vector.tensor_mul(tmp[0:32, :], raw[32:64, :], sa[32:64, sl])
                nc.vector.tensor_mul(tmp[32:64, :], raw[32:64, :], ca[32:64, sl])
                nc.vector.tensor_mul(dst[0:32, dst_sl], raw[0:32, :], ca[0:32, sl])
                nc.vector.tensor_mul(dst[32:64, dst_sl], raw[0:32, :], sa[0:32, sl])
                nc.vector.tensor_sub(
                    dst[0:32, dst_sl], dst[0:32, dst_sl], tmp[0:32, :]
                )
                nc.vector.tensor_add(
                    dst[32:64, dst_sl], dst[32:64, dst_sl], tmp[32:64, :]
                )

            # ---------------- phase B/C: up-projections + attention ---------
            with (
                tc.tile_pool(name="pw2", bufs=1) as pw2,
                tc.tile_pool(name="ph", bufs=1) as ph,
                tc.tile_pool(name="pat", bufs=1) as pat,
            ):
                # ---------------- phase A: cos/sin, x^T + down-proj by c-strip --
                with (
                    tc.tile_pool(name="pa", bufs=1) as pa,
                    tc.tile_pool(name="pw", bufs=1) as pw,
                ):
                    # ca/sa = [cos; cos], [sin; sin] transposed to [64, T]
                    for s in range(T // P):
                        for ext, dst, tg in ((cos_ext, ca, "cstrip"),
                                             (sin_ext, sa, "sstrip")):
                            strip = pa.tile([P, RHD // 2], F32R, tag=tg, bufs=2,
                                            name=tg)
                            nc.sync.dma_start(strip[:], ext.ap()[s * P:(s + 1) * P, :])
                            tp = ptp.tile([P, P], F32R, tag="tp", name="tp")
                            nc.tensor.transpose(tp[: RHD // 2, :], strip[:], ident[:])
                            nc.vector.tensor_copy(dst[0:32, s * P:(s + 1) * P],
                                                  tp[:32, :])
                            nc.vector.tensor_copy(dst[32:64, s * P:(s + 1) * P],
                                                  tp[:32, :])

                    kr_raw = pa.tile([RHD, TCH], F32, tag="kr_raw",
                                     name="kr_raw")
                    cq_part = [pa.tile([P, TCH], F32, tag=f"cqp{i}",
                                       name=f"cqp{i}") for i in range(NNL)]
                    ckv_part = [pa.tile([P, TCH], F32, tag=f"ckvp{i}",
                                        name=f"ckvp{i}") for i in range(NNL)]

                    for co in range(NCS):        # 512-wide strip of C
                        c0 = co * TCH
                        # transposed weight strips for this c-strip
                        wdqTs = [pw.tile([P, NL], F32R, tag=f"wdqT{i}",
                                         name=f"wdqT{i}") for i in range(4)]
                        wdkvTs = [pw.tile([P, NL], F32R, tag=f"wdkvT{i}",
                                          name=f"wdkvT{i}") for i in range(4)]
                        for w_ext, wTs in ((wdq_ext, wdqTs), (wdkv_ext, wdkvTs)):
                            for rp in range(NL // P // 2):
                                stripA = pw.tile([P, TCH], F32R, tag="wstripA",
                                                 bufs=2, name="wstripA")
                                stripB = pw.tile([P, TCH], F32R, tag="wstripB",
                                                 bufs=2, name="wstripB")
                                nc.sync.dma_start(
                                    stripA[:],
                                    w_ext.ap()[2 * rp * P:(2 * rp + 1) * P, c0:c0 + TCH],
                                )
                                nc.sync.dma_start(
                                    stripB[:],
                                    w_ext.ap()[(2 * rp + 1) * P:(2 * rp + 2) * P, c0:c0 + TCH],
                                )
                                for ci in range(4):
                                    transpose_pair_into(
                                        wTs[ci][:, 2 * rp * P:(2 * rp + 2) * P],
                                        stripA[:, ci * P:(ci + 1) * P],
                                        stripB[:, ci * P:(ci + 1) * P],
                                        eng="act",
                                    )
                        wkrTs = [pw.tile([P, RHD], F32R, tag=f"wkrT{i}",
                                         name=f"wkrT{i}") for i in range(4)]
                        kstrip = pw.tile([RHD, TCH], F32R, tag="kstrip",
                                         name="kstrip")
                        nc.sync.dma_start(kstrip[:], wkr_ext.ap()[:, c0:c0 + TCH])
                        for ci in range(4):
                            tp = ptp.tile([P, P], F32R, tag="tp", name="tp")
                            nc.tensor.transpose(
                                tp[:, :RHD], kstrip[:, ci * P:(ci + 1) * P],
                                ident[:RHD, :RHD],
                            )
                            ev, od = _deint(tp[:, :RHD])
                            nc.scalar.copy(wkrTs[ci][:, 0:32], ev)
                            nc.scalar.copy(wkrTs[ci][:, 32:64], od)

                        # x^T for this c-strip (this core's 512-row T-chunk only)
                        xTs = [pa.tile([P, TCH], F32R, tag=f"xt{i}",
                                       name=f"xt{i}") for i in range(4)]
                        for tp_ in range(TCH // P // 2):
                            xnA = pa.tile([P, TCH], F32R, tag="xnA", bufs=2,
                                          name="xnA")
                            xnB = pa.tile([P, TCH], F32R, tag="xnB", bufs=2,
                                          name="xnB")
                            nc.sync.dma_start(
                                xnA[:],
                                x_ext.ap()[2 * tp_ * P:(2 * tp_ + 1) * P, c0:c0 + TCH],
                            )
                            nc.sync.dma_start(
                                xnB[:],
                                x_ext.ap()[(2 * tp_ + 1) * P:(2 * tp_ + 2) * P, c0:c0 + TCH],
                            )
                            for ci in range(4):
                                transpose_pair_into(
                                    xTs[ci][:, 2 * tp_ * P:(2 * tp_ + 2) * P],
                                    xnA[:, ci * P:(ci + 1) * P],
                                    xnB[:, ci * P:(ci + 1) * P],
                                )

                        # partial down projections, accumulated across c-strips
                        for wTs, dstP in ((wdqTs, cq_part), (wdkvTs, ckv_part)):
                            for nl in range(NNL):
                                acc = pmm.tile([P, TCH], F32, tag="mm", name="mm")
                                for ci in range(4):
                                    nc.tensor.matmul(
                                        acc[:],
                                        wTs[ci][:, nl * P:(nl + 1) * P],
                                        xTs[ci][:],
                                        start=(ci == 0),
                                        stop=(ci == 3),
                                    )
                                if co == 0:
                                    nc.vector.tensor_copy(dstP[nl][:], acc[:])
                                else:
                                    nc.vector.tensor_add(
                                        dstP[nl][:], dstP[nl][:], acc[:]
                                    )
                        acc = pmm.tile([RHD, TCH], F32, tag="mm", name="mm")
                        for ci in range(4):
                            nc.tensor.matmul(
                                acc[:],
                                wkrTs[ci][:],
                                xTs[ci][:],
                                start=(ci == 0),
                                stop=(ci == 3),
                            )
                        if co == 0:
                            nc.vector.tensor_copy(kr_raw[:], acc[:])
                        else:
                            nc.vector.tensor_add(kr_raw[:], kr_raw[:], acc[:])

                    # ship partials: [cq(512); ckv(512); kr(64)] x TCH
                    for nl in range(NNL):
                        nc.gpsimd.dma_start(
                            out=agin_dram.ap()[nl * P:(nl + 1) * P, :],
                            in_=cq_part[nl][:],
                        )
                        nc.gpsimd.dma_start(
                            out=agin_dram.ap()[NL + nl * P:NL + (nl + 1) * P, :],
                            in_=ckv_part[nl][:],
                        )
                    nc.gpsimd.dma_start(out=agin_dram.ap()[2 * NL:2 * NL + RHD, :],
                                        in_=kr_raw[:])
                    nc.gpsimd.collective_compute(
                        "AllGather",
                        mybir.AluOpType.bypass,
                        replica_groups=[[0, 1, 2, 3], [4, 5, 6, 7]],
                        ins=[agin_dram.ap().opt()],
                        outs=[agout_dram.ap().opt()],
                    )
                    wuqT = [pw2.tile([P, HLOC * HS], BF16, tag=f"wuqT{i}",
                                     name=f"wuqT{i}") for i in range(NNL)]
                    wukT = [pw2.tile([P, HLOC * HS], BF16, tag=f"wukT{i}",
                                     name=f"wukT{i}") for i in range(NNL)]
                    wuvT = [pw2.tile([P, HLOC * HS], BF16, tag=f"wuvT{i}",
                                     name=f"wuvT{i}") for i in range(NNL)]
                    for w_ext, wT in ((wuq_ext, wuqT), (wuk_ext, wukT),
                                      (wuv_ext, wuvT)):
                        for rp in range(HLOC * HS // P // 2):
                            stripA = pw2.tile([P, NL], F32R, tag="usA",
                                              bufs=2, name="usA")
                            stripB = pw2.tile([P, NL], F32R, tag="usB",
                                              bufs=2, name="usB")
                            nc.sync.dma_start(
                                stripA[:],
                                w_ext.ap()[2 * rp * P:(2 * rp + 1) * P, :],
                            )
                            nc.sync.dma_start(
                                stripB[:],
                                w_ext.ap()[(2 * rp + 1) * P:(2 * rp + 2) * P, :],
                            )
                            for cs in range(NNL):
                                transpose_pair_into(
                                    wT[cs][:, 2 * rp * P:(2 * rp + 2) * P],
                                    stripA[:, cs * P:(cs + 1) * P],
                                    stripB[:, cs * P:(cs + 1) * P],
                                    eng="act",
                                )
                    wqrT = [pw2.tile([P, HLOC * RHD], BF16, tag=f"wqrT{i}",
                                     name=f"wqrT{i}") for i in range(NNL)]
                    for rs in range(HLOC * RHD // P):
                        strip = pw2.tile([P, NL], F32R, tag="ustrip", bufs=2,
                                         name="ustrip")
                        nc.sync.dma_start(strip[:], wqr_ext.ap()[rs * P:(rs + 1) * P, :])
                        for cs in range(NNL):
                            tp = ptp.tile([P, P], F32R, tag="tp", name="tp")
                            nc.tensor.transpose(
                                tp[:], strip[:, cs * P:(cs + 1) * P], ident[:]
                            )
                            for hh in range(2):
                                hloc = rs * 2 + hh
                                ev, od = _deint(tp[:, hh * RHD:(hh + 1) * RHD])
                                base = hloc * RHD
                                nc.scalar.copy(
                                    wqrT[cs][:, base:base + 32], ev
                                )
                                nc.scalar.copy(
                                    wqrT[cs][:, base + 32:base + 64], od
                                )

                    # transpose W_o during the collective window, staged
                    # to DRAM for phase D
                    for sp in range(C // P // 2):
                        osA = pw.tile([P, HLOC * HS], F32R, tag="osA",
                                      bufs=1, name="osA")
                        osB = pw.tile([P, HLOC * HS], F32R, tag="osB",
                                      bufs=1, name="osB")
                        nc.sync.dma_start(
                            osA[:],
                            wo_ext.ap()[2 * sp * P:(2 * sp + 1) * P, :],
                        )
                        nc.sync.dma_start(
                            osB[:],
                            wo_ext.ap()[(2 * sp + 1) * P:(2 * sp + 2) * P, :],
                        )
                        for fs in range(HLOC):
                            tp2 = ptp.tile([P, 2 * P], F32R, tag="tp",
                                           name="tp")
                            nc.tensor.transpose(
                                tp2[:, 0:P], osA[:, fs * P:(fs + 1) * P],
                                ident[:],
                            )
                            nc.tensor.transpose(
                                tp2[:, P:2 * P], osB[:, fs * P:(fs + 1) * P],
                                ident[:],
                            )
                            wob = pw.tile([P, 2 * P], BF16, tag="wob",
                                          bufs=2, name="wob")
                            nc.scalar.copy(wob[:], tp2[:])
                            nc.sync.dma_start(
                                woT_dram.ap()[fs, :,
                                              2 * sp * P:(2 * sp + 2) * P],
                                wob[:],
                            )

                    # unpack gathered latents into [feat, T] layout
                    for ch in range(NCH):
                        sl = slice(ch * TCH, (ch + 1) * TCH)
                        for nl in range(NNL):
                            nc.sync.dma_start(
                                cqT[nl][:, sl],
                                agout_dram.ap()[ch, nl * P:(nl + 1) * P, :],
                            )
                            nc.sync.dma_start(
                                ckvT[nl][:, sl],
                                agout_dram.ap()[ch, NL + nl * P:NL + (nl + 1) * P, :],
                            )
                        krg = pa.tile([RHD, TCH], BF16, tag="krg", bufs=2,
                                      name="krg")
                        nc.sync.dma_start(
                            krg[:], agout_dram.ap()[ch, 2 * NL:2 * NL + RHD, :]
                        )
                        tmp = pa.tile([RHD, TCH], F32, tag="rtmp", bufs=1,
                                      name="rtmp")
                        rope(kr, sl, krg[:], tmp, sl)

                for h in range(HLOC):
                    qcT = ph.tile([P, T], F32R, tag="qcT", name="qcT")
                    kcT = ph.tile([P, T], F32R, tag="kcT", name="kcT")
                    qr = ph.tile([RHD, T], F32R, tag="qr", name="qr")
                    vv = ph.tile([P, T], F32R, tag="vv", name="vv")
                    hs = slice(h * P, (h + 1) * P)
                    for ch in range(NCH):
                        sl = slice(ch * TCH, (ch + 1) * TCH)
                        for wT, srcT, dst in (
                            (wuqT, cqT, qcT),
                            (wukT, ckvT, kcT),
                        ):
                            acc = pmm.tile([P, TCH], F32, tag="mm", name="mm")
                            for nl in range(NNL):
                                nc.tensor.matmul(
                                    acc[:],
                                    wT[nl][:, hs],
                                    srcT[nl][:, sl],
                                    start=(nl == 0),
                                    stop=(nl == NNL - 1),
                                )
                            nc.vector.tensor_copy(dst[:, sl], acc[:])
                        # q_r raw + rope
                        acc = pmm.tile([RHD, TCH], F32, tag="mm", name="mm")
                        for nl in range(NNL):
                            nc.tensor.matmul(
                                acc[:],
                                wqrT[nl][:, h * RHD:(h + 1) * RHD],
                                cqT[nl][:, sl],
                                start=(nl == 0),
                                stop=(nl == NNL - 1),
                            )
                        tmp = ph.tile([RHD, TCH], F32, tag="rtmp2", name="rtmp2")
                        rope(qr, sl, acc[:], tmp, sl)
                    # v: compute v^T [hs, t] then PE-transpose to natural
                    for ch in range(NCH):
                        sl = slice(ch * TCH, (ch + 1) * TCH)
                        acc = pmm.tile([P, TCH], F32, tag="mm", name="mm")
                        for nl in range(NNL):
                            nc.tensor.matmul(
                                acc[:],
                                wuvT[nl][:, hs],
                                ckvT[nl][:, sl],
                                start=(nl == 0),
                                stop=(nl == NNL - 1),
                            )
                        vts = ph.tile([P, TCH], F32R, tag="vts", bufs=2,
                                      name="vts")
                        nc.scalar.copy(vts[:], acc[:])
                        for sp in range(2):
                            tt = ch * 4 + 2 * sp
                            transpose_pair_into(
                                vv[:, tt * P:(tt + 2) * P],
                                vts[:, 2 * sp * P:(2 * sp + 1) * P],
                                vts[:, (2 * sp + 1) * P:(2 * sp + 2) * P],
                                eng="act",
                            )

                    # ---- causal attention for this head ----
                    for tq in range(NCH):
                        outU = pou.tile([P, TCH], F32, tag="ou", name="ou")
                        den = pou.tile([1, TCH], F32, tag="de", name="de")
                        nkc = tq + 1
                        qsl = slice(tq * TCH, (tq + 1) * TCH)
                        for kc in range(nkc):
                            for ks in range(4):
                                kt = kc * 4 + ks
                                k0 = kt * P
                                ST = pmm.tile([P, TCH], F32, tag="mm",
                                              name="mm")
                                nc.tensor.matmul(
                                    ST[:],
                                    kcT[:, k0:k0 + P],
                                    qcT[:, qsl],
                                    start=True,
                                    stop=False,
                                )
                                nc.tensor.matmul(
                                    ST[:],
                                    kr[:, k0:k0 + P],
                                    qr[:, qsl],
                                    start=False,
                                    stop=True,
                                )
                                if kc == tq:
                                    off = 384 - ks * P
                                    nc.vector.tensor_add(
                                        ST[:], ST[:],
                                        maskbuf[:, off:off + TCH],
                                    )
                                Pt = pat.tile([P, TCH], F32R, tag="pt",
                                              bufs=6, name="pt")
                                nc.scalar.activation(Pt[:], ST[:], Exp,
                                                     scale=SCALE)
                                last = kc == nkc - 1 and ks == 3
                                first = kc == 0 and ks == 0
                                nc.tensor.matmul(
                                    den[:],
                                    onesb[:, 0:1],
                                    Pt[:],
                                    start=first,
                                    stop=last,
                                    skip_group_check=True,
                                )
                                nc.tensor.matmul(
                                    outU[:],
                                    vv[:, k0:k0 + P],
                                    Pt[:],
                                    start=first,
                                    stop=last,
                                    skip_group_check=True,
                                )
                        recip = pat.tile([1, TCH], F32, tag="rc", name="rc")
                        nc.vector.reciprocal(recip[:], den[:])
                        recipr = pat.tile([1, TCH], F32R, tag="rcr", name="rcr")
                        nc.vector.tensor_copy(recipr[:], recip[:])
                        bc = pmm.tile([P, TCH], F32, tag="mm", name="mm")
                        nc.tensor.matmul(
                            bc[:], onesb[0:1, :], recipr[:],
                            start=True, stop=True,
                        )
                        bc_sb = pat.tile([P, TCH], F32, tag="bcs", bufs=2,
                                         name="bcs")
                        nc.scalar.activation(bc_sb[:], bc[:], Copy)
                        oh = pat.tile([P, TCH], BF16, tag="oh", bufs=2,
                                      name="oh")
                        nc.vector.tensor_mul(oh[:], outU[:], bc_sb[:])
                        nc.sync.dma_start(
                            ahT_dram.ap()[h, :, tq * TCH:(tq + 1) * TCH], oh[:]
                        )

            # ---------------- phase D: output projection --------------------
            with tc.tile_pool(name="pd", bufs=1) as pd:
                woT = [pd.tile([P, C], BF16, tag=f"woT{i}", name=f"woT{i}")
                       for i in range(HLOC)]
                for fs in range(HLOC):
                    nc.sync.dma_start(woT[fs][:], woT_dram.ap()[fs])
                for tq in range(NCH):
                    ah = []
                    for h in range(HLOC):
                        t = pd.tile([P, TCH], BF16, tag=f"ah{h}", bufs=2,
                                    name=f"ah{h}")
                        nc.sync.dma_start(
                            t[:], ahT_dram.ap()[h, :, tq * TCH:(tq + 1) * TCH]
                        )
                        ah.append(t)
                    for cs in range(C // P):
                        acc = pmm.tile([P, TCH], F32, tag="mm", name="mm")
                        for h in range(HLOC):
                            nc.tensor.matmul(
                                acc[:],
                                woT[h][:, cs * P:(cs + 1) * P],
                                ah[h][:],
                                start=(h == 0),
                                stop=(h == HLOC - 1),
                            )
                        ot = pd.tile([P, TCH], F32, tag="ot", bufs=3, name="ot")
                        nc.scalar.copy(ot[:], acc[:])
                        nc.sync.dma_start(
                            out_ext.ap()[cs * P:(cs + 1) * P,
                                         tq * TCH:(tq + 1) * TCH],
                            ot[:],
                        )

    nc.compile()
    return nc


def _get_nc():
    if "nc" not in _NC_CACHE:
        _NC_CACHE["nc"] = build()
    return _NC_CACHE["nc"]


def kernel(x, freqs_cos, freqs_sin, W_dq, W_uq, W_dkv, W_uk, W_uv, W_qr, W_kr,
           W_o, trace=False, **trace_kwargs):
    nc = _get_nc()
    f32 = lambda a: np.ascontiguousarray(np.asarray(a, dtype=np.float32))
    x = f32(x); W_dq = f32(W_dq); W_uq = f32(W_uq); W_dkv = f32(W_dkv)
    W_uk = f32(W_uk); W_uv = f32(W_uv); W_qr = f32(W_qr); W_kr = f32(W_kr)
    W_o = f32(W_o)
    cos = f32(freqs_cos); sin = f32(freqs_sin)

    in_maps = []
    for c in range(8):
        b, r = divmod(c, 4)
        in_maps.append({
            "x": x[b, r * TCH:(r + 1) * TCH],
            "wdq": W_dq, "wdkv": W_dkv, "wkr": W_kr,
            "wuq": W_uq[r * HLOC * HS:(r + 1) * HLOC * HS],
            "wuk": W_uk[r * HLOC * HS:(r + 1) * HLOC * HS],
            "wuv": W_uv[r * HLOC * HS:(r + 1) * HLOC * HS],
            "wqr": W_qr[r * HLOC * RHD:(r + 1) * HLOC * RHD],
            "wo": W_o[:, r * HLOC * HS:(r + 1) * HLOC * HS],
            "cos": cos, "sin": sin,
        })
    res = run_bass_kernel_spmd(nc, in_maps, core_ids=list(range(8)),
                               trace=trace, **trace_kwargs)
    out = np.zeros((B, T, C), dtype=np.float32)
    for c in range(8):
        b = c // 4
        out[b] += res.results[c]["out"].T
    kernel.last_result = res
    return out



# revision 55
# speedup vs baseline: 2.0760x; 2.0760x over previous
"""MLA-style attention kernel for 8 TRN2 NeuronCores (v3).

Sharding: core c -> batch b = c//4, heads r*4..r*4+3 where r = c%4.
Each core computes cq for its own T-chunk (AllGathered in fp8 across the
4-core group and consumed directly by fp8 DoubleRow Q up-projections),
ckv/kr for the FULL T redundantly (hides the collective), its 4 heads'
attention, and a partial output projection summed on the host.

All transposes are done on the HOST: every weight arrives pre-transposed
(rope weights pre-permuted to planar re/im layout) in bf16, x arrives as
x^T in bf16, so the kernel contains no PE transposes.  V is computed
directly in natural [t, (head, hs)] layout by swapping matmul operand
roles.  Q/K score operands are stored as packed fp8e4 [content | rope]
pairs ([128, 2T] with zero-padded rope rows) so each S^T tile is ONE
DoubleRow matmul contracting 256 dims at half cycles/row; the fp8 scale
(16 per side) is folded into the exp scale.  Q-side up-projection
weights are fp8 (host-scaled by 64 out of the subnormal range) and
contract the gathered fp8 cq with DoubleRow matmuls; the scale unwinds
in the PSUM->fp8 convert.  The probability/value path stays bf16.  Causal softmax runs on S^T tiles [k, q]; diagonal blocks
restrict exp/den/PV to the valid column range and apply a 128-wide
additive boundary mask.  Denominators accumulate via a ones-column
matmul; the reciprocal is broadcast with gpsimd partition_broadcast and
applied on DVE.  W_o for chunk tq is deferred one iteration so the last
head's normalization chain hides under the next chunk's attention.

Scheduling: DMA issue order is arranged so the critical loads (wdq/xo
stripes, wdkv stripes, xT halves) win the FIFO DMA engines early, the
fp8 agin lands right after cq, and bulk loads follow; PE is kept warm
through the initial DMA window by dummy matmuls; per-chunk q_r rope work
is software-pipelined into the attention loop so the in-order DVE queue
never blocks normalization; early-needed tiles (cq staging) live in the
outer pool to avoid SBUF write-after-read waits on the phase-A region.
"""
import math
import numpy as np
import ml_dtypes

import concourse.bass as bass
import concourse.bacc as bacc
import concourse.mybir as mybir
import concourse.tile as tile
from concourse.bass_utils import run_bass_kernel_spmd

F32 = mybir.dt.float32
BF16 = mybir.dt.bfloat16
FP8 = mybir.dt.float8e4
QKS = 16.0            # fp8 scale for q/k operands
WUS = 64.0            # fp8 scale for Q-side up-proj weights
Exp = mybir.ActivationFunctionType.Exp
Copy = mybir.ActivationFunctionType.Copy

B, T, C = 2, 2048, 2048
H = 16
HS = 128
NL = 512
RHD = 64
HLOC = 4              # heads per core
P = 128
NNL = NL // P         # 4 latent p-tiles
TCH = 512
NCH = T // TCH        # 4 T-chunks
NCT = C // P          # 16 contraction p-tiles over C
SCALE = 1.0 / math.sqrt(HS + RHD)
SCALE8 = SCALE / (QKS * QKS)
NEG = -1.0e30

_NC_CACHE = {}
_PREP_CACHE = {}


def build():
    nc = bacc.Bacc("TRN2", target_bir_lowering=False, debug=False, num_devices=8)

    xT_ext = nc.dram_tensor("xT", [C, T], BF16, kind="ExternalInput")
    xo_ext = nc.dram_tensor("xo", [C, TCH], BF16, kind="ExternalInput")
    wdqT_ext = nc.dram_tensor("wdqT", [C, NL], BF16, kind="ExternalInput")
    wdkvT_ext = nc.dram_tensor("wdkvT", [C, NL], BF16, kind="ExternalInput")
    wkrT_ext = nc.dram_tensor("wkrT", [C, RHD], BF16, kind="ExternalInput")
    wuqT_ext = nc.dram_tensor("wuqT", [NL, HLOC * HS], FP8, kind="ExternalInput")
    wukT_ext = nc.dram_tensor("wukT", [NL, HLOC * HS], BF16, kind="ExternalInput")
    wuvT_ext = nc.dram_tensor("wuvT", [NL, HLOC * HS], BF16, kind="ExternalInput")
    wqrT_ext = nc.dram_tensor("wqrT", [NL, HLOC * RHD], FP8, kind="ExternalInput")
    woT_ext = nc.dram_tensor("woT", [HLOC * HS, C], BF16, kind="ExternalInput")
    caT_ext = nc.dram_tensor("caT", [P, T], BF16, kind="ExternalInput")
    saT_ext = nc.dram_tensor("saT", [P, T], BF16, kind="ExternalInput")
    out_ext = nc.dram_tensor("out", [C, T], BF16, kind="ExternalOutput")

    ones_dram = nc.inline_tensor(np.ones((P, P), dtype=ml_dtypes.bfloat16),
                                 name="onesc")
    # boundary mask for the diagonal 128-col sub-block of S^T tiles [k, q]:
    # m2[jj, u] = 0 if u >= jj else -1e30
    m2 = np.full((P, P), NEG, dtype=ml_dtypes.bfloat16)
    for jj in range(P):
        m2[jj, jj:] = 0.0
    m2_dram = nc.inline_tensor(m2, name="m2c")

    agin_dram = nc.dram_tensor("agin", [NL, TCH], FP8)
    agout_dram = nc.dram_tensor("agout", [NCH, NL, TCH], FP8)

    with tile.TileContext(nc) as tc:
        with (
            tc.tile_pool(name="pers", bufs=1) as pers,
            tc.tile_pool(name="pmm", bufs=4, space="PSUM") as pmm,
            tc.tile_pool(name="pou", bufs=2, space="PSUM") as pou,
        ):
            ones = pers.tile([P, P], BF16, tag="ones", name="ones")
            m2b = pers.tile([P, P], BF16, tag="m2b", name="m2b")

            ca = pers.tile([P, T], BF16, tag="ca", name="ca")
            sa = pers.tile([P, T], BF16, tag="sa", name="sa")
            # PE p-state warmup: dummy matmuls on not-yet-loaded tiles keep the
            # tensor engine continuously busy through the initial DMA wait so
            # real matmuls start at full clock.
            for wi in range(14):
                warm = pmm.tile([P, TCH], F32, tag="mm", name="mm")
                nc.tensor.matmul(
                    warm[:], ca[:, 0:P], sa[:, 0:TCH],
                    start=True, stop=True, skip_group_check=True,
                )

            # persistent activations
            ckv_sb = pers.tile([P, NNL * T], BF16, tag="ckv", name="ckv")
            kr = pers.tile([RHD, T], BF16, tag="kr", name="kr")
            cq8 = pers.tile([P, NNL * T], FP8, tag="cq8", name="cq8")

            # up/out-projection weights (preloaded early, used later)
            wuq_sb = pers.tile([P, NNL * HLOC * HS], FP8, tag="wuq", name="wuq")
            wuk_sb = pers.tile([P, NNL * HLOC * HS], BF16, tag="wuk", name="wuk")
            wuv_sb = pers.tile([P, NNL * HLOC * HS], BF16, tag="wuv", name="wuv")
            wqr_sb = pers.tile([P, NNL * HLOC * RHD], FP8, tag="wqr", name="wqr")
            wo_sb = pers.tile([P, HLOC * C], BF16, tag="wo", name="wo")

            def rope(dst0, dst32, raw, rb, tmp, sl):
                """Planar rope on bf16 SBUF operands (DVE 2x mode).
                dst0/dst32 = 32-row re/im output APs (base-0); raw = staged
                bf16 tile, band at partition rb; tmp = [64, w] bf16 at base 0.
                Tables ca/sa are 4x-planar [128, T]."""
                nc.vector.tensor_mul(tmp[0:32, :], raw[rb + 32:rb + 64, :], sa[rb + 32:rb + 64, sl])
                nc.vector.tensor_mul(tmp[32:64, :], raw[rb + 32:rb + 64, :], ca[rb + 32:rb + 64, sl])
                nc.vector.tensor_mul(dst0, raw[rb:rb + 32, :], ca[rb:rb + 32, sl])
                nc.vector.tensor_mul(dst32, raw[rb:rb + 32, :], sa[rb:rb + 32, sl])
                nc.vector.tensor_sub(dst0, dst0, tmp[0:32, :])
                nc.vector.tensor_add(dst32, dst32, tmp[32:64, :])

            # ---------------- phase A: down-projections ----------------
            if True:
                with tc.tile_pool(name="pa", bufs=1) as pa:
                    wdq_sb = pa.tile([P, NCT * NL // NNL * NNL], BF16,
                                     tag="wdq", name="wdq")
                    wdkv_sb = pa.tile([P, NCT * NL], BF16, tag="wdkv",
                                      name="wdkv")
                    wkr_sb = pa.tile([P, NCT * RHD], BF16, tag="wkr",
                                     name="wkr")
                    xo_sb = pa.tile([P, NCT * TCH], BF16, tag="xo", name="xo")
                    for st in range(4):
                        nc.sync.dma_start(
                            wdq_sb[:, st * 4 * NL:(st + 1) * 4 * NL].rearrange(
                                "p (a w) -> p a w", a=4),
                            wdqT_ext.ap()[st * 4 * P:(st + 1) * 4 * P, :].rearrange(
                                "(a p) w -> p a w", p=P),
                        )
                        nc.sync.dma_start(
                            xo_sb[:, st * 4 * TCH:(st + 1) * 4 * TCH].rearrange(
                                "p (a w) -> p a w", a=4),
                            xo_ext.ap()[st * 4 * P:(st + 1) * 4 * P, :].rearrange(
                                "(a p) w -> p a w", p=P),
                        )
                    for st in range(4):
                        nc.sync.dma_start(
                            wdkv_sb[:, st * 4 * NL:(st + 1) * 4 * NL].rearrange(
                                "p (a w) -> p a w", a=4),
                            wdkvT_ext.ap()[st * 4 * P:(st + 1) * 4 * P, :].rearrange(
                                "(a p) w -> p a w", p=P),
                        )
                    nc.sync.dma_start(
                        wkr_sb[:].rearrange("p (a w) -> p a w", a=NCT),
                        wkrT_ext.ap().rearrange("(a p) w -> p a w", p=P),
                    )
                    xT = [pa.tile([P, T], BF16, tag=f"xT{i}", name=f"xT{i}")
                          for i in range(NCT)]
                    for ct in range(NCT):
                        nc.gpsimd.dma_start(
                            out=xT[ct][:, 0:2 * TCH],
                            in_=xT_ext.ap()[ct * P:(ct + 1) * P, 0:2 * TCH],
                        )

                    # cq for the own T-chunk only -> AllGather
                    cq_own = pa.tile([P, NNL * TCH], FP8, tag="cqo",
                                     name="cqo")
                    cqacc = [pmm.tile([P, TCH], F32, tag="mm", name="mm")
                             for _ in range(NNL)]
                    for ct in range(NCT):
                        for ot in range(NNL):
                            nc.tensor.matmul(
                                cqacc[ot][:],
                                wdq_sb[:, ct * NL + ot * P:ct * NL + (ot + 1) * P],
                                xo_sb[:, ct * TCH:(ct + 1) * TCH],
                                start=(ct == 0),
                                stop=(ct == NCT - 1),
                            )
                    for ot in range(NNL):
                        nc.scalar.activation(
                            cq_own[:, ot * TCH:(ot + 1) * TCH], cqacc[ot][:],
                            Copy, scale=QKS,
                        )
                    nc.sync.dma_start(
                        agin_dram.ap().rearrange("(a p) w -> p a w", p=P),
                        cq_own[:].rearrange("p (a w) -> p a w", a=NNL),
                    )
                    nc.gpsimd.collective_compute(
                        "AllGather",
                        mybir.AluOpType.bypass,
                        replica_groups=[[0, 1, 2, 3], [4, 5, 6, 7]],
                        ins=[agin_dram.ap().opt()],
                        outs=[agout_dram.ap().opt()],
                    )

                    for ct in range(NCT):
                        nc.sync.dma_start(
                            xT[ct][:, 2 * TCH:],
                            xT_ext.ap()[ct * P:(ct + 1) * P, 2 * TCH:],
                        )
                    nc.gpsimd.dma_start(out=ones[:], in_=ones_dram.ap())
                    nc.gpsimd.dma_start(out=m2b[:], in_=m2_dram.ap())
                    nc.sync.dma_start(ca[:], caT_ext.ap())
                    nc.sync.dma_start(sa[:], saT_ext.ap())
                    # ckv + kr for the FULL T (hides the collective)
                    for ch in range(NCH):
                        tsl = slice(ch * TCH, (ch + 1) * TCH)
                        if ch == 0:
                            # ct-outer: 4 concurrent PSUM groups consume x/w
                            # tiles in DMA-arrival order (no per-group rescan)
                            accs = [pmm.tile([P, TCH], F32, tag="mm",
                                             name="mm") for _ in range(NNL)]
                            for ct in range(NCT):
                                for ot in range(NNL):
                                    nc.tensor.matmul(
                                        accs[ot][:],
                                        wdkv_sb[:, ct * NL + ot * P:ct * NL + (ot + 1) * P],
                                        xT[ct][:, tsl],
                                        start=(ct == 0),
                                        stop=(ct == NCT - 1),
                                    )
                            for ot in range(NNL):
                                nc.vector.tensor_copy(
                                    ckv_sb[:, ot * T + ch * TCH:ot * T + (ch + 1) * TCH],
                                    accs[ot][:],
                                )
                        else:
                            for ot in range(NNL):
                                acc = pmm.tile([P, TCH], F32, tag="mm", name="mm")
                                for ct in range(NCT):
                                    nc.tensor.matmul(
                                        acc[:],
                                        wdkv_sb[:, ct * NL + ot * P:ct * NL + (ot + 1) * P],
                                        xT[ct][:, tsl],
                                        start=(ct == 0),
                                        stop=(ct == NCT - 1),
                                    )
                                nc.vector.tensor_copy(
                                    ckv_sb[:, ot * T + ch * TCH:ot * T + (ch + 1) * TCH],
                                    acc[:],
                                )
                        acck = pmm.tile([RHD, TCH], F32, tag="mm", name="mm")
                        for ct in range(NCT):
                            nc.tensor.matmul(
                                acck[:],
                                wkr_sb[:, ct * RHD:(ct + 1) * RHD],
                                xT[ct][:, tsl],
                                start=(ct == 0),
                                stop=(ct == NCT - 1),
                            )
                        krst = pa.tile([RHD, TCH], BF16, tag="krst",
                                       bufs=2, name="krst")
                        nc.vector.tensor_copy(krst[:], acck[:])
                        tmp = pa.tile([RHD, TCH], BF16, tag="rtmp", bufs=2,
                                      name="rtmp")
                        rope(kr[0:32, tsl], kr[32:64, tsl], krst, 0, tmp, tsl)

            # ------------- phase B: up-projections (K, V, Q) -------------
            with (
                tc.tile_pool(name="ph", bufs=1) as ph,
                tc.tile_pool(name="pat", bufs=1) as pat,
            ):
                # packed fp8 score operands: cols [0:T] = content (d=128),
                # cols [T:2T] = rope (rows 0:64, rows 64:128 zeroed) -> one
                # DoubleRow matmul contracts both halves at half cycles/row.
                qk8 = [ph.tile([P, 2 * T], FP8, tag=f"qk8{i}", name=f"qk8{i}")
                       for i in range(HLOC)]
                kk8 = [ph.tile([P, 2 * T], FP8, tag=f"kk8{i}", name=f"kk8{i}")
                       for i in range(HLOC)]
                for i in range(HLOC):
                    nc.gpsimd.memset(qk8[i][RHD:P, T:2 * T], 0.0)
                    nc.gpsimd.memset(kk8[i][RHD:P, T:2 * T], 0.0)
                qr = [ph.tile([RHD, T], BF16, tag=f"qr{i}", name=f"qr{i}")
                      for i in range(HLOC)]
                vv = ph.tile([P, (T // P) * TCH], BF16, tag="vv", name="vv")

                # Q content (needs the gathered cq)
                for nl in range(NNL):
                    for ch in range(NCH):
                        nc.sync.dma_start(
                            cq8[:, nl * T + ch * TCH:nl * T + (ch + 1) * TCH],
                            agout_dram.ap()[ch, nl * P:(nl + 1) * P, :],
                        )
                for ext, sb in ((wuqT_ext, wuq_sb), (wqrT_ext, wqr_sb),
                                (wuvT_ext, wuv_sb), (wukT_ext, wuk_sb)):
                    nc.sync.dma_start(
                        sb[:].rearrange("p (a w) -> p a w", a=NNL),
                        ext.ap().rearrange("(a p) w -> p a w", p=P),
                    )
                nc.sync.dma_start(
                    wo_sb[:].rearrange("p (a w) -> p a w", a=HLOC),
                    woT_ext.ap().rearrange("(a p) w -> p a w", p=P),
                )
                # Q rope: packed 2 heads per matmul, rope'd to base-0 strips.
                # Chunk ch is produced just-in-time: ch0 up front, ch(tq+1)
                # pipelined inside the attention loop so DVE rope work for
                # later chunks overlaps attention instead of queueing ahead
                # of its normalization ops.
                def qr_chunk(ch):
                    tsl = slice(ch * TCH, (ch + 1) * TCH)
                    wqrv = wqr_sb[:].rearrange("p (a w) -> p a w", a=NNL)
                    cq8v2 = cq8[:].rearrange("p (a w) -> p a w", a=NNL)
                    for pr in range(HLOC // 2):
                        acc = pmm.tile([P, TCH], F32, tag="mm", name="mm")
                        for pr2 in range(NNL // 2):
                            nc.tensor.matmul(
                                acc[:],
                                wqrv[:, 2 * pr2:2 * pr2 + 2, pr * P:(pr + 1) * P],
                                cq8v2[:, 2 * pr2:2 * pr2 + 2, ch * TCH:(ch + 1) * TCH],
                                start=(pr2 == 0),
                                stop=(pr2 == NNL // 2 - 1),
                                perf_mode=mybir.MatmulPerfMode.DoubleRow,
                            )
                        qst = pat.tile([P, TCH], BF16, tag="qst", bufs=2,
                                       name="qst")
                        nc.vector.tensor_copy(qst[:], acc[:])
                        for sub in range(2):
                            h = pr * 2 + sub
                            tmp = pat.tile([RHD, TCH], BF16, tag="rtmp2",
                                           bufs=2, name="rtmp2")
                            rope(qr[h][0:32, tsl], qr[h][32:64, tsl],
                                 qst, sub * RHD, tmp, tsl)
                            nc.scalar.activation(
                                qk8[h][0:RHD, T + ch * TCH:T + (ch + 1) * TCH],
                                qr[h][:, tsl], Copy, scale=QKS / (QKS * WUS),
                            )


                # ------------- phase C/D: attention + output proj -------------

                qr_chunk(0)
                wuqv = wuq_sb[:].rearrange("p (a w) -> p a w", a=NNL)
                cq8v = cq8[:].rearrange("p (a w) -> p a w", a=NNL)
                for h in range(HLOC):
                    for ch in range(NCH):
                        tsl = slice(ch * TCH, (ch + 1) * TCH)
                        acc = pmm.tile([P, TCH], F32, tag="mm", name="mm")
                        for pr2 in range(NNL // 2):
                            nc.tensor.matmul(
                                acc[:],
                                wuqv[:, 2 * pr2:2 * pr2 + 2, h * P:(h + 1) * P],
                                cq8v[:, 2 * pr2:2 * pr2 + 2, tsl],
                                start=(pr2 == 0),
                                stop=(pr2 == NNL // 2 - 1),
                                perf_mode=mybir.MatmulPerfMode.DoubleRow,
                            )
                        nc.scalar.activation(
                            qk8[h][:, tsl], acc[:], Copy, scale=QKS / (QKS * WUS),
                        )
                # V directly in natural [t, (h, hs)] layout
                for tb in range(T // P):
                    acc = pmm.tile([P, HLOC * HS], F32, tag="mm", name="mm")
                    for nl in range(NNL):
                        nc.tensor.matmul(
                            acc[:],
                            ckv_sb[:, nl * T + tb * P:nl * T + (tb + 1) * P],
                            wuv_sb[:, nl * HLOC * HS:(nl + 1) * HLOC * HS],
                            start=(nl == 0),
                            stop=(nl == NNL - 1),
                        )
                    nc.vector.tensor_copy(vv[:, tb * TCH:(tb + 1) * TCH],
                                          acc[:])
                for i in range(HLOC):
                    for ch in range(NCH):
                        tsl = slice(ch * TCH, (ch + 1) * TCH)
                        nc.scalar.activation(
                            kk8[i][0:RHD, T + ch * TCH:T + (ch + 1) * TCH],
                            kr[:, tsl], Copy, scale=QKS,
                        )
                for h in range(HLOC):
                    hs = slice(h * P, (h + 1) * P)
                    for ch in range(NCH):
                        tsl = slice(ch * TCH, (ch + 1) * TCH)
                        acc = pmm.tile([P, TCH], F32, tag="mm", name="mm")
                        for nl in range(NNL):
                            nc.tensor.matmul(
                                acc[:],
                                wuk_sb[:, nl * HLOC * HS + h * P:nl * HLOC * HS + (h + 1) * P],
                                ckv_sb[:, nl * T + ch * TCH:nl * T + (ch + 1) * TCH],
                                start=(nl == 0),
                                stop=(nl == NNL - 1),
                            )
                        nc.scalar.activation(
                            kk8[h][:, tsl], acc[:], Copy, scale=QKS,
                        )
                def w_o_chunk(tq, ohs, dve_ot=False):
                    qsl2 = slice(tq * TCH, (tq + 1) * TCH)
                    for cs in range(C // P):
                        acc = pmm.tile([P, TCH], F32, tag="mm", name="mm")
                        for h in range(HLOC):
                            nc.tensor.matmul(
                                acc[:],
                                wo_sb[:, h * C + cs * P:h * C + (cs + 1) * P],
                                ohs[h][:],
                                start=(h == 0),
                                stop=(h == HLOC - 1),
                            )
                        ot = pat.tile([P, TCH], BF16, tag="ot", bufs=3,
                                      name="ot")
                        if dve_ot:
                            nc.vector.tensor_copy(ot[:], acc[:])
                        else:
                            nc.scalar.copy(ot[:], acc[:])
                        nc.sync.dma_start(
                            out_ext.ap()[cs * P:(cs + 1) * P, qsl2],
                            ot[:],
                        )

                oh_prev = None
                for tq in range(NCH):
                    qsl = slice(tq * TCH, (tq + 1) * TCH)
                    oh_cur = [pat.tile([P, TCH], BF16, tag=f"oh{i}",
                                       name=f"oh{i}", bufs=2)
                              for i in range(HLOC)]
                    for h in range(HLOC):
                        outU = pou.tile([P, TCH], F32, tag="ou", name="ou")
                        den = pou.tile([1, TCH], F32, tag="de", name="de")
                        # 1-tile software pipeline: emit ST(i+1) before
                        # den/PV(i) so the exp latency hides under the next
                        # score matmul instead of stalling the PE stream.
                        def emit_dp(p):
                            Ptp, offp, firstp, lastp, ktp = p
                            nc.tensor.matmul(
                                den[:, offp:],
                                ones[:, 0:1],
                                Ptp[:, offp:],
                                start=firstp,
                                stop=lastp,
                                skip_group_check=True,
                            )
                            nc.tensor.matmul(
                                outU[:, offp:],
                                vv[:, ktp * TCH + h * P:ktp * TCH + (h + 1) * P],
                                Ptp[:, offp:],
                                start=firstp,
                                stop=lastp,
                                skip_group_check=True,
                            )

                        prev = None
                        for kc in range(tq + 1):
                            diag = kc == tq
                            for ks in range(4):
                                kt = kc * 4 + ks
                                k0 = kt * P
                                off = ks * P if diag else 0
                                ST = pmm.tile([P, TCH], F32, tag="mm",
                                              name="mm")
                                kkv = kk8[h][:].rearrange(
                                    "p (a t) -> p a t", a=2)
                                qkv = qk8[h][:].rearrange(
                                    "p (a t) -> p a t", a=2)
                                nc.tensor.matmul(
                                    ST[:, off:],
                                    kkv[:, :, k0:k0 + P],
                                    qkv[:, :, tq * TCH + off:(tq + 1) * TCH],
                                    start=True,
                                    stop=True,
                                    perf_mode=mybir.MatmulPerfMode.DoubleRow,
                                )
                                if diag:
                                    nc.vector.tensor_add(
                                        ST[:, off:off + P], ST[:, off:off + P],
                                        m2b[:],
                                    )
                                Pt = pat.tile([P, TCH], BF16, tag="pt",
                                              bufs=6, name="pt")
                                nc.scalar.activation(Pt[:, off:], ST[:, off:],
                                                     Exp, scale=SCALE8)
                                if prev is not None:
                                    emit_dp(prev)
                                prev = (Pt, off, kc == 0 and ks == 0,
                                        kc == tq and ks == 3, kt)
                        emit_dp(prev)
                        recipb = pat.tile([1, TCH], BF16, tag="rcb", name="rcb",
                                          bufs=2)
                        with nc.allow_low_precision(reason="softmax recip fits bf16"):
                            nc.vector.reciprocal(recipb[:], den[:])
                        bcast = pat.tile([P, TCH], BF16, tag="bcs", bufs=2,
                                         name="bcs")
                        nc.gpsimd.partition_broadcast(bcast[:], recipb[:])
                        nc.vector.tensor_mul(oh_cur[h][:], outU[:], bcast[:])
                    if tq + 1 < NCH:
                        qr_chunk(tq + 1)
                    if tq > 0:
                        w_o_chunk(tq - 1, oh_prev, dve_ot=(tq >= 3))
                    oh_prev = oh_cur
                w_o_chunk(NCH - 1, oh_prev, dve_ot=True)

    nc.compile()
    return nc


def _get_nc():
    if "nc" not in _NC_CACHE:
        _NC_CACHE["nc"] = build()
    return _NC_CACHE["nc"]


def _planar_perm(d):
    # [0, 2, 4, ..., d-2, 1, 3, ..., d-1]
    return np.concatenate([np.arange(0, d, 2), np.arange(1, d, 2)])


def kernel(x, freqs_cos, freqs_sin, W_dq, W_uq, W_dkv, W_uk, W_uv, W_qr, W_kr,
           W_o, trace=False, **trace_kwargs):
    nc = _get_nc()
    bf = ml_dtypes.bfloat16
    f8 = ml_dtypes.float8_e4m3fn
    cT8 = lambda a: np.ascontiguousarray(
        (np.asarray(a, dtype=np.float32).T * 64.0).astype(f8))
    f32 = lambda a: np.asarray(a, dtype=np.float32)
    cT = lambda a: np.ascontiguousarray(f32(a).T.astype(bf))

    x = f32(x)
    cos = f32(freqs_cos)
    sin = f32(freqs_sin)

    # host-side preprocessing (shared across cores)
    wdqT = cT(W_dq)                       # [C, NL]
    wdkvT = cT(W_dkv)                     # [C, NL]
    perm_r = _planar_perm(RHD)
    wkrT = cT(f32(W_kr)[perm_r])          # [C, RHD] planar
    # rope tables: 4x-planar duplicated [128, T]
    caT = np.ascontiguousarray(
        np.tile(cos.T, (4, 1)).astype(bf))  # [128, T]
    saT = np.ascontiguousarray(
        np.tile(sin.T, (4, 1)).astype(bf))
    xTb = [np.ascontiguousarray(x[b].T.astype(bf)) for b in range(B)]

    W_qr_f = f32(W_qr)
    in_maps = []
    for c in range(8):
        b, r = divmod(c, 4)
        hsl = slice(r * HLOC * HS, (r + 1) * HLOC * HS)
        # planar-permute W_qr rows per local head
        wqr_rows = []
        for hh in range(HLOC):
            base = r * HLOC * RHD + hh * RHD
            wqr_rows.append(W_qr_f[base:base + RHD][perm_r])
        wqrT = np.ascontiguousarray(
            (np.concatenate(wqr_rows, axis=0).T * 64.0).astype(f8))  # [NL, 256]
        in_maps.append({
            "xT": xTb[b],
            "xo": np.ascontiguousarray(xTb[b][:, r * TCH:(r + 1) * TCH]),
            "wdqT": wdqT, "wdkvT": wdkvT, "wkrT": wkrT,
            "wuqT": cT8(f32(W_uq)[hsl]),
            "wukT": cT(f32(W_uk)[hsl]),
            "wuvT": cT(f32(W_uv)[hsl]),
            "wqrT": wqrT,
            "woT": cT(f32(W_o)[:, hsl]),
            "caT": caT, "saT": saT,
        })
    res = run_bass_kernel_spmd(nc, in_maps, core_ids=list(range(8)),
                               trace=trace, **trace_kwargs)
    out = np.zeros((B, T, C), dtype=np.float32)
    for c in range(8):
        b = c // 4
        out[b] += res.results[c]["out"].astype(np.float32).T
    kernel.last_result = res
    return out
